# revision 19
# baseline (speedup 1.0000x reference)
"""Trainium2 Bass kernel for entmax-1.5 via n-section (nn_EntmaxNsect).

Full input: X [4096, 32000] f32 -> output entmax weights, same shape.
Data-parallel over 8 NeuronCores (512 rows each), row-blocks of 128 per core.

Math (validated offline vs the jax reference):
  alpha=1.5 -> p = relu(0.5x - tau)^2 / Z with tau from a 5x5-section search.
  Equivalent: snap the root tau* of f(tau)=sum relu(0.5x-tau)^2 - 1 onto the
  fp32 lattice the reference builds, replaying its interval arithmetic.
  tau* from a Newton ladder (coarse chunkmax-128, fine chunkmax-8, then one
  full-data Newton round). Z from the local quadratic expansion; rsqrt(Z) via
  2 Newton iters (Z ~= 1 always). Output p = Square(Relu(0.5*s*x - tau*s)).

Raw-bass, explicit engines + semaphores. d() = engine drain for same-engine
pipeline hazards (required by the HW model: consecutive engine ops overlap).
"""

import contextlib
import numpy as np

P = 128
D = 32000
W8 = 8
N8 = D // W8          # 4000
W128 = 16
N128 = N8 // W128     # 250
NQ = 4
QW = D // NQ          # 8000
CW = 2000
NCH = D // CW         # 16
CPQ = QW // CW        # 4
C_HI = 0.005590169943749474
COARSE_ITERS = 4
FINE_ITERS = 2
NY = 2
NO2 = 2


def _bc(small, like):
    from concourse.bass import broadcast_tensor_aps
    return broadcast_tensor_aps(small, like)[0]


def build_entmax_kernel(nc, n_rows):
    import concourse.mybir as mybir
    f32 = mybir.dt.float32
    x = nc.dram_tensor("X", [n_rows, D], f32, kind="ExternalInput")
    out = nc.dram_tensor("OUT", [n_rows, D], f32, kind="ExternalOutput")
    return build_entmax_kernel_aps(nc, x[:, :], out[:, :], n_rows)


def build_entmax_kernel_aps(nc, x, out, n_rows):
    import concourse.mybir as mybir

    f32 = mybir.dt.float32
    AX = mybir.AxisListType.X
    OP = mybir.AluOpType
    AF = mybir.ActivationFunctionType

    nblk = n_rows // P
    assert n_rows % P == 0

    ctx = contextlib.ExitStack()
    with ctx:
        _n = [0]

        def sb(shape):
            _n[0] += 1
            return ctx.enter_context(nc.sbuf_tensor(f"t{_n[0]}", shape, f32))

        xq = [sb([P, QW]) for _ in range(NQ)]
        m8 = sb([P, N8])
        m128 = sb([P, N128])
        ybuf = [sb([P, CW]) for _ in range(NY)]
        tr = sb([P, CW])
        o1 = sb([P, CW])
        o2 = [sb([P, CW]) for _ in range(NO2)]
        kf4 = sb([P, 4])

        def sm2(n=1):
            return [sb([P, n]) for _ in range(2)]
        mxs, lo0, th, th2 = sm2(), sm2(), sm2(), sm2()
        Bp, Ap = sm2(NCH), sm2(NCH)
        Nhp, Nh = sm2(2), sm2()
        Bx, Axm, Fs, den, rr = sm2(), sm2(), sm2(), sm2(), sm2()
        lo, hi, wd, tks, cm4, cd = sm2(), sm2(), sm2(), sm2(4), sm2(4), sm2()
        tau, dd, zt, dsq, Z, zh, y1, u, sv = (
            sm2(), sm2(), sm2(), sm2(), sm2(), sm2(), sm2(), sm2(), sm2())
        sc_t, bi_t = sm2(), sm2()
        ones = sb([P, 1])
        lB, lF = sb([P, 2]), sb([P, 2])
        lBs, lFs, lnum, lden, lstep, lrec = (sb([P, 1]) for _ in range(6))
        mxx = sb([P, 1])

        s_load = [ctx.enter_context(nc.semaphore(f"s_load{j}"))
                  for j in range(NQ)]
        s_y = ctx.enter_context(nc.semaphore("s_y"))
        s_sq = ctx.enter_context(nc.semaphore("s_sq"))
        s_bias = ctx.enter_context(nc.semaphore("s_bias"))
        s_o2 = ctx.enter_context(nc.semaphore("s_o2"))
        s_od = [ctx.enter_context(nc.semaphore(f"s_od{j}"))
                for j in range(NO2)]
        s_blk = ctx.enter_context(nc.semaphore("s_blk"))

        block = ctx.enter_context(nc.Block())

        @block.sync
        def _(sp):
            for b in range(nblk):
                r0 = b * P
                for q in range(NQ):
                    if b > 0:
                        sp.wait_ge(s_o2, NCH * (b - 1) + CPQ * q + CPQ)
                    sp.dma_start(
                        xq[q][:], x[r0:r0 + P, q * QW:(q + 1) * QW]
                    ).then_inc(s_load[q], 16)
                for c in range(NCH):
                    oc = NCH * b + c
                    sp.wait_ge(s_o2, oc + 1)
                    col = c * CW
                    sp.dma_start(
                        out[r0:r0 + P, col:col + CW], o2[oc % NO2][:]
                    ).then_inc(s_od[oc % NO2], 16)

        @block.vector
        def _(dve):
            d = dve.drain
            dve.memset(ones[:], 1.0)
            dve.memset(kf4[:, 0:1], 1.0)
            dve.memset(kf4[:, 1:2], 2.0)
            dve.memset(kf4[:, 2:3], 3.0)
            dve.memset(kf4[:, 3:4], 4.0)
            for b in range(nblk):
                i = b % 2
                npq = N8 // NQ
                for q in range(NQ):
                    dve.wait_ge(s_load[q], 16 * (b + 1))
                    dve.tensor_reduce(
                        m8[:, q * npq:(q + 1) * npq],
                        xq[q][:].rearrange("p (c w) -> p c w", w=W8),
                        axis=AX, op=OP.max)
                d()
                dve.tensor_reduce(
                    m128[:], m8[:].rearrange("p (c w) -> p c w", w=W128),
                    axis=AX, op=OP.max)
                d()
                dve.tensor_reduce(mxx[:], m128[:], axis=AX, op=OP.max)
                d()
                dve.tensor_scalar_mul(mxs[i][:], mxx[:], 0.5)
                d()
                dve.tensor_scalar_add(lo0[i][:], mxs[i][:], -1.0)
                dve.tensor_scalar_add(th[i][:], mxs[i][:], -0.5)
                d()
                dve.tensor_scalar_mul(th2[i][:], th[i][:], 2.0)
                d()

                def ladder(vals, width, nparts, clamp):
                    pw = width // nparts
                    for k in range(nparts):
                        yk = ybuf[k % NY]
                        dve.scalar_tensor_tensor(
                            yk[:, :pw], vals[:, k * pw:(k + 1) * pw],
                            th2[i][:], _bc(th2[i][:], yk[:, :pw]),
                            op0=OP.max, op1=OP.subtract,
                            accum_out=lB[:, k:k + 1])
                        d()
                        dve.scalar_tensor_tensor(
                            yk[:, :pw], yk[:, :pw], 1.0, yk[:, :pw],
                            op0=OP.mult, op1=OP.mult,
                            accum_out=lF[:, k:k + 1])
                        d()
                    if nparts > 1:
                        dve.tensor_reduce(lBs[:], lB[:], axis=AX, op=OP.add)
                        dve.tensor_reduce(lFs[:], lF[:], axis=AX, op=OP.add)
                        d()
                        B_, F_ = lBs, lFs
                    else:
                        B_, F_ = lB, lF
                    dve.tensor_scalar(lnum[:], F_[:, 0:1], 0.25, -1.0,
                                      op0=OP.mult, op1=OP.add)
                    dve.tensor_scalar_max(lden[:], B_[:, 0:1], 1e-20)
                    d()
                    dve.reciprocal(lrec[:], lden[:])
                    d()
                    dve.tensor_mul(lstep[:], lnum[:], lrec[:])
                    d()
                    dve.tensor_add(th[i][:], th[i][:], lstep[:])
                    d()
                    if clamp:
                        dve.tensor_max(th[i][:], th[i][:], lo0[i][:])
                        d()
                        dve.tensor_tensor(th[i][:], th[i][:], mxs[i][:],
                                          op=OP.min)
                        d()
                    dve.tensor_scalar_mul(th2[i][:], th[i][:], 2.0)
                    d()

                for it in range(COARSE_ITERS):
                    ladder(m128[:], N128, 1, clamp=(it == COARSE_ITERS - 1))
                for it in range(FINE_ITERS):
                    ladder(m8[:], N8, 2, clamp=True)

                # full-data eval round (independent chunks; no drains needed
                # between them, slot WAR handled via s_sq waits)
                for c in range(NCH):
                    g = NCH * b + c
                    if g >= NY:
                        dve.wait_ge(s_sq, g - NY + 1)
                    q, cc = divmod(c, CPQ)
                    dve.scalar_tensor_tensor(
                        ybuf[g % NY][:], xq[q][:, cc * CW:(cc + 1) * CW],
                        th2[i][:], _bc(th2[i][:], ybuf[g % NY][:]),
                        op0=OP.max, op1=OP.subtract,
                        accum_out=Bp[i][:, c:c + 1]
                    ).then_inc(s_y, 1)
                # wait for ACT eval squares; ybuf free after this
                dve.wait_ge(s_sq, NCH * (b + 1))
                for k in range(2):
                    dve.scalar_tensor_tensor(
                        ybuf[k][:], m8[:, k * CW:(k + 1) * CW],
                        th2[i][:], _bc(ones[:], ybuf[k][:]),
                        op0=OP.is_gt, op1=OP.mult,
                        accum_out=Nhp[i][:, k:k + 1])
                d()
                dve.tensor_reduce(Nh[i][:], Nhp[i][:], axis=AX, op=OP.add)
                dve.tensor_reduce(Bx[i][:], Bp[i][:], axis=AX, op=OP.add)
                dve.tensor_reduce(Axm[i][:], Ap[i][:], axis=AX, op=OP.add)
                d()
                dve.tensor_scalar(Fs[i][:], Axm[i][:], 0.25, -1.0,
                                  op0=OP.mult, op1=OP.add)
                dve.tensor_scalar_max(den[i][:], Bx[i][:], 1e-20)
                d()
                dve.reciprocal(lrec[:], den[i][:])
                d()
                dve.tensor_mul(rr[i][:], Fs[i][:], lrec[:])
                d()
                dve.tensor_add(rr[i][:], rr[i][:], th[i][:])
                d()
                dve.tensor_max(rr[i][:], rr[i][:], lo0[i][:])
                d()
                dve.tensor_tensor(rr[i][:], rr[i][:], mxs[i][:], op=OP.min)
                # snap replay (lo/hi serial chain)
                dve.tensor_copy(lo[i][:], lo0[i][:])
                dve.tensor_scalar_add(hi[i][:], mxs[i][:],
                                      float(-np.float32(C_HI)))
                d()
                for it in range(5):
                    dve.tensor_sub(wd[i][:], hi[i][:], lo[i][:])
                    d()
                    dve.tensor_scalar_mul(wd[i][:], wd[i][:], 0.2)
                    d()
                    dve.scalar_tensor_tensor(
                        tks[i][:], kf4[:], wd[i][:],
                        _bc(lo[i][:], tks[i][:]), op0=OP.mult, op1=OP.add)
                    d()
                    dve.scalar_tensor_tensor(
                        cm4[i][:], tks[i][:], rr[i][:],
                        _bc(ones[:], cm4[i][:]),
                        op0=OP.is_le, op1=OP.mult,
                        accum_out=cd[i][:])
                    d()
                    dve.scalar_tensor_tensor(
                        lo[i][:], cd[i][:], wd[i][:], lo[i][:],
                        op0=OP.mult, op1=OP.add)
                    d()
                    dve.tensor_add(hi[i][:], lo[i][:], wd[i][:])
                    d()
                dve.tensor_add(tau[i][:], lo[i][:], hi[i][:])
                d()
                dve.tensor_scalar_mul(tau[i][:], tau[i][:], 0.5)
                d()
                # Z and rsqrt
                dve.tensor_sub(dd[i][:], tau[i][:], th[i][:])
                d()
                dve.tensor_mul(zt[i][:], dd[i][:], Bx[i][:])
                dve.tensor_mul(dsq[i][:], dd[i][:], dd[i][:])
                d()
                dve.tensor_sub(zt[i][:], Fs[i][:], zt[i][:])
                d()
                dve.scalar_tensor_tensor(
                    zt[i][:], dsq[i][:], Nh[i][:], zt[i][:],
                    op0=OP.mult, op1=OP.add)
                d()
                dve.tensor_scalar_add(Z[i][:], zt[i][:], 1.0)
                d()
                dve.tensor_scalar_max(Z[i][:], Z[i][:], 0.5)
                d()
                dve.tensor_scalar_min(Z[i][:], Z[i][:], 2.0)
                d()
                dve.tensor_scalar_mul(zh[i][:], Z[i][:], 0.5)
                d()
                dve.tensor_scalar(y1[i][:], zh[i][:], -1.0, 1.5,
                                  op0=OP.mult, op1=OP.add)
                d()
                dve.tensor_mul(u[i][:], y1[i][:], y1[i][:])
                d()
                dve.tensor_mul(u[i][:], u[i][:], zh[i][:])
                d()
                dve.tensor_scalar(u[i][:], u[i][:], -1.0, 1.5,
                                  op0=OP.mult, op1=OP.add)
                d()
                dve.tensor_mul(sv[i][:], y1[i][:], u[i][:])
                d()
                dve.tensor_scalar_mul(sc_t[i][:], sv[i][:], 0.5)
                dve.tensor_scalar(bi_t[i][:], tau[i][:], sv[i][:], -1.0,
                                  op0=OP.mult, op1=OP.mult)
                d()
                dve.nop().then_inc(s_bias, 1)
                dve.nop().then_inc(s_blk, 1)

        @block.scalar
        def _(act):
            for b in range(nblk):
                i = b % 2
                for c in range(NCH):
                    g = NCH * b + c
                    if b > 0 and c == 0:
                        act.wait_ge(s_blk, b)
                    act.wait_ge(s_y, g + 1)
                    act.activation(ybuf[g % NY][:], ybuf[g % NY][:],
                                   AF.Square,
                                   accum_out=Ap[i][:, c:c + 1]
                                   ).then_inc(s_sq, 1)
                act.wait_ge(s_bias, b + 1)
                for c in range(NCH):
                    oc = NCH * b + c
                    q, cc = divmod(c, CPQ)
                    if oc >= NO2:
                        act.wait_ge(s_od[oc % NO2], 16 * (oc // NO2))
                    act.activation(o1[:], xq[q][:, cc * CW:(cc + 1) * CW],
                                   AF.Relu, bias=bi_t[i][:], scale=sc_t[i][:])
                    act.drain()
                    act.activation(o2[oc % NO2][:], o1[:], AF.Square
                                   ).then_inc(s_o2, 1)
                    act.drain()

    return nc


_CACHE = {}


def _get_nc(n_rows):
    if n_rows in _CACHE:
        return _CACHE[n_rows]
    import concourse.bass as bass

    nc = bass.Bass("TRN2")
    build_entmax_kernel(nc, n_rows)
    nc.finalize()
    _CACHE[n_rows] = nc
    return nc


def kernel(X: np.ndarray) -> np.ndarray:
    from concourse.bass_utils import run_bass_kernel_spmd

    n_cores = 8
    rows = X.shape[0]
    shard = rows // n_cores
    X = np.ascontiguousarray(X, dtype=np.float32)
    nc = _get_nc(shard)
    in_maps = [{"X": X[i * shard:(i + 1) * shard]} for i in range(n_cores)]
    res = run_bass_kernel_spmd(nc, in_maps, core_ids=list(range(n_cores)))
    return np.concatenate([r["OUT"] for r in res.results], axis=0)
